# revision 37
# baseline (speedup 1.0000x reference)
"""Trainium2 Bass kernel for nn_BM2_15822659518813 (dense_cnn).

Pipeline per sample (B=32 sharded 4-per-core across 8 cores):
  x2u = DynConv1x1(x2; u2)              # 128->128 on 64x64
  l   = DynConv1x1(x3; u1)              # 256->128 on 32x32
  lr  = cat(x2u, upsample2x(l))         # 256ch, 64x64   (never materialized)
  b   = CA(lr)                          # channel mask, folded into dl1 weights
  out = DynConv1x1(b; dl1)              # 256->128 on 64x64

Key tricks:
  - per-sample dynamic 1x1 kernels (aw) built on DVE via scalar_tensor_tensor
  - CA mask folded into dl1 aw rows (no full-tensor multiply)
  - nearest 2x upsample via 0-step replicated matmul rhs APs (free on PE)
  - avg-pools fused into PSUM->SBUF copies via ACT accum_out
  - sigmoid via exp (stays in one ACT table set)
  - softmax partition-broadcast via tiny DRAM bounce
"""

import sys

if "/opt/trn_rl_repo" not in sys.path:
    sys.path.insert(0, "/opt/trn_rl_repo")

import numpy as np
import ml_dtypes

import concourse.bacc as bacc
import concourse.bass as bass
import concourse.tile as tile
import concourse.mybir as mybir
from concourse.bass_utils import run_bass_kernel_spmd

F32 = mybir.dt.float32
BF16 = mybir.dt.bfloat16
AFT = mybir.ActivationFunctionType
OP = mybir.AluOpType

N_CORES = 8
B = 32
BL = B // N_CORES          # 4 samples per core
C1 = 128
C2 = 256
K = 4
HW2 = 64 * 64              # 4096
HW3 = 32 * 32              # 1024
TEMP = 34.0

CDT = BF16                 # compute dtype for matmul operands
DEBUG = False              # emit debug DRAM outputs (sim only)
REPEAT = 1                 # >1: wrap body in a HW loop (timing builds only)
ABLATE = ""                # "", "dmaonly", "noatt" (timing diagnostics)
AWD_GP = False             # build dl1 aw chunk 1 on GPSIMD (parallel to DVE)


def _ap(t, offset_extra, dims):
    return bass.AP(tensor=t.tensor, offset=t.offset + offset_extra, ap=dims)


def build_nc():
    nc = bacc.Bacc("TRN2", target_bir_lowering=False, debug=False)

    # ---------- DRAM I/O ----------
    x2 = nc.dram_tensor("x2", [BL, C1, HW2], CDT, kind="ExternalInput")
    x3 = nc.dram_tensor("x3", [BL, 2, 128, HW3], CDT, kind="ExternalInput")
    y = nc.dram_tensor("y", [BL, C1, HW2], F32, kind="ExternalOutput")

    # params (host pre-transposed; see kernel() for layouts)
    u2_wT = nc.dram_tensor("u2_wT", [1, 128, K, C1], CDT, kind="ExternalInput")
    u1_wT = nc.dram_tensor("u1_wT", [2, 128, K, C1], CDT, kind="ExternalInput")
    dl1_wT = nc.dram_tensor("dl1_wT", [2, 128, K, C1], CDT, kind="ExternalInput")
    u2_bT = nc.dram_tensor("u2_bT", [C1, K], F32, kind="ExternalInput")
    u1_bT = nc.dram_tensor("u1_bT", [C1, K], F32, kind="ExternalInput")
    dl1_bT = nc.dram_tensor("dl1_bT", [C1, K], F32, kind="ExternalInput")
    # fc1 lhsT: [c_chunks, 128, hid_pad]; fc2 rhs: [hid_chunks, 128, K]
    u2_fc1T = nc.dram_tensor("u2_fc1T", [1, 128, 256], F32, kind="ExternalInput")
    u1_fc1T = nc.dram_tensor("u1_fc1T", [2, 128, 384], F32, kind="ExternalInput")
    dl1_fc1T = nc.dram_tensor("dl1_fc1T", [2, 128, 384], F32, kind="ExternalInput")
    u2_fc2T = nc.dram_tensor("u2_fc2T", [2, 128, K], F32, kind="ExternalInput")
    u1_fc2T = nc.dram_tensor("u1_fc2T", [3, 128, K], F32, kind="ExternalInput")
    dl1_fc2T = nc.dram_tensor("dl1_fc2T", [3, 128, K], F32, kind="ExternalInput")
    fc2_b = nc.dram_tensor("fc2_b", [4, 3 * K], F32, kind="ExternalInput")  # [b<=4, (set,k)]
    ca_w1T = nc.dram_tensor("ca_w1T", [2, 128, C1], F32, kind="ExternalInput")
    ca_w2T = nc.dram_tensor("ca_w2T", [128, C2], F32, kind="ExternalInput")
    ca_b1 = nc.dram_tensor("ca_b1", [C1, 1], F32, kind="ExternalInput")
    ca_b2 = nc.dram_tensor("ca_b2", [2, 128], F32, kind="ExternalInput")

    dbg = {}
    if DEBUG:
        dbg = {
            "d_avgx2": nc.dram_tensor("d_avgx2", [128, BL], F32, kind="ExternalOutput"),
            "d_att01": nc.dram_tensor("d_att01", [128, BL, 2, K], F32, kind="ExternalOutput"),
            "d_mask": nc.dram_tensor("d_mask", [128, 2, BL], F32, kind="ExternalOutput"),
            "d_V": nc.dram_tensor("d_V", [128, 2, 2, BL], F32, kind="ExternalOutput"),
            "d_x2u0": nc.dram_tensor("d_x2u0", [128, HW2], CDT, kind="ExternalOutput"),
            "d_l0": nc.dram_tensor("d_l0", [128, HW3], CDT, kind="ExternalOutput"),
            "d_aw2_0": nc.dram_tensor("d_aw2_0", [128, C1], CDT, kind="ExternalOutput"),
            "d_awd0": nc.dram_tensor("d_awd0", [128, 2, C1], CDT, kind="ExternalOutput"),
            "d_attdl": nc.dram_tensor("d_attdl", [128, BL, 1, K], F32, kind="ExternalOutput"),
            "d_abdl": nc.dram_tensor("d_abdl", [128, BL], F32, kind="ExternalOutput"),
        }
    with tile.TileContext(nc) as tc:
        _emit(nc, tc, {**locals(), **dbg})
    nc.compile()
    return nc


def _emit(nc, tc, T):
    import contextlib

    ctx = contextlib.ExitStack()
    with ctx:
        if REPEAT > 1:
            ctx.enter_context(
                tc.For_i(0, REPEAT, 1, hint_engines=tuple(mybir.ALL_ENGINES))
            )
        par = ctx.enter_context(tc.tile_pool(name="par", bufs=1))
        stats = ctx.enter_context(tc.tile_pool(name="stats", bufs=1))
        xin = ctx.enter_context(tc.tile_pool(name="xin", bufs=4))
        x3in = ctx.enter_context(tc.tile_pool(name="x3in", bufs=4))
        keep = ctx.enter_context(tc.tile_pool(name="keep", bufs=1))
        outp = ctx.enter_context(tc.tile_pool(name="outp", bufs=3))
        awp = ctx.enter_context(tc.tile_pool(name="awp", bufs=1))
        attp = ctx.enter_context(tc.tile_pool(name="attp", bufs=2))
        bigps = ctx.enter_context(tc.tile_pool(name="bigps", bufs=2, space="PSUM"))
        smps = ctx.enter_context(tc.tile_pool(name="smps", bufs=4, space="PSUM"))
        drp = ctx.enter_context(tc.tile_pool(name="drp", bufs=2, space="DRAM"))

        # ---------- load params ----------
        def ld(dram, shape, transpose=None, dtype=None):
            t = par.tile(shape, dtype or dram.ap().dtype, tag=dram.ap().name)
            src = dram.ap()
            if transpose:
                src = src.transpose(transpose)
            nc.sync.dma_start(t, src)
            return t

        p_u2w = ld(T["u2_wT"], [128, 1, K, C1], [1, 0, 2, 3])
        p_u1w = ld(T["u1_wT"], [128, 2, K, C1], [1, 0, 2, 3])
        p_dlw = ld(T["dl1_wT"], [128, 2, K, C1], [1, 0, 2, 3])
        p_u2b = ld(T["u2_bT"], [C1, K])
        p_u1b = ld(T["u1_bT"], [C1, K])
        p_dlb = ld(T["dl1_bT"], [C1, K])
        p_u2f1 = ld(T["u2_fc1T"], [128, 1, 256], [1, 0, 2])
        p_u1f1 = ld(T["u1_fc1T"], [128, 2, 384], [1, 0, 2])
        p_dlf1 = ld(T["dl1_fc1T"], [128, 2, 384], [1, 0, 2])
        p_u2f2 = ld(T["u2_fc2T"], [128, 2, K], [1, 0, 2])
        p_u1f2 = ld(T["u1_fc2T"], [128, 3, K], [1, 0, 2])
        p_dlf2 = ld(T["dl1_fc2T"], [128, 3, K], [1, 0, 2])
        p_f2b = ld(T["fc2_b"], [4, 3 * K])
        p_cw1 = ld(T["ca_w1T"], [128, 2, C1], [1, 0, 2])
        p_cw2 = ld(T["ca_w2T"], [128, C2])
        p_cb1 = ld(T["ca_b1"], [C1, 1])
        p_cb2 = ld(T["ca_b2"], [128, 2], [1, 0])

        # ---------- stats tiles ----------
        sum_x2 = stats.tile([128, BL], F32, tag="sum_x2")      # raw sums of x2
        avg_x3 = stats.tile([128, 2, BL], F32, tag="avg_x3")
        xu_part = stats.tile([128, 4, BL], F32, tag="xu_part")  # x2u partial sums
        V = stats.tile([128, 2, 2, BL], F32, tag="V")           # [c-chunk, avg/max, s]
        lsum = stats.tile([128, BL], F32, tag="lsum")
        avg_x2 = stats.tile([128, BL], F32, tag="avg_x2")
        mask = stats.tile([128, 2, BL], F32, tag="mask")
        pooled_dl = stats.tile([128, 2, BL], F32, tag="pooled_dl")

        # ---------- per-sample input DMA + input pooling ----------
        X2 = []
        X3 = []
        for s in range(BL):
            t2 = xin.tile([128, HW2], CDT, tag="x2")
            nc.sync.dma_start(t2, T["x2"].ap()[s, :, :])
            t3 = x3in.tile([128, 2, HW3], CDT, tag="x3")
            nc.sync.dma_start(t3, T["x3"].ap()[s, :, :, :].transpose([1, 0, 2]))
            X2.append(t2)
            X3.append(t3)
            if ABLATE:
                continue
            nc.vector.reduce_sum(sum_x2[:, s : s + 1], t2, axis=mybir.AxisListType.X)
            nc.vector.reduce_sum(avg_x3[:, :, s], t3, axis=mybir.AxisListType.X)
        if ABLATE == "dmaonly":
            for s in range(BL):
                ot = outp.tile([128, HW2], F32, tag="out")
                v = X2[s].bitcast(F32)
                nc.sync.dma_start(T["y"].ap()[s, :, 0 : HW2 // 2], v)
                nc.sync.dma_start(T["y"].ap()[s, :, HW2 // 2 : HW2], v)
            return
        if not ABLATE:
            nc.vector.tensor_scalar_mul(avg_x2, sum_x2, 1.0 / HW2)
            nc.vector.tensor_scalar_mul(avg_x3, avg_x3, 1.0 / HW3)

        # ---------- helpers (group-parameterized) ----------
        GS = 4                      # samples per group
        NG = BL // GS

        def att_mlp(fc1T, ncs, nh, pooled, tag):
            """fc1T: [128, ncs, hidpad]; pooled sliced to GS columns."""
            h = attp.tile([128, nh, GS], F32, tag=tag)
            for m in range(nh):
                hp = smps.tile([128, GS], F32, tag="sm")
                for c in range(ncs):
                    rhs = pooled[:, c, :] if ncs > 1 else pooled
                    nc.tensor.matmul(
                        hp, fc1T[:, c, 128 * m : 128 * (m + 1)], rhs,
                        start=(c == 0), stop=(c == ncs - 1),
                    )
                nc.scalar.activation(h[:, m, :], hp, AFT.Relu)
            return h

        def att_fc2(h, fc2T, nh, lg_ps):
            for m in range(nh):
                nc.tensor.matmul(
                    lg_ps, h[:, m, :], fc2T[:, m, :],
                    start=(m == 0), stop=(m == nh - 1),
                )

        def exp_bcast(lg_sb, nsets, tag):
            """lg_sb: [GS, nsets*K] logits+bias -> (e, r): unnormalized
            exp [128, GS, nsets, K] and 1/sum [128, GS, nsets]. Consumers
            build aw/ab from e (no wait on the sum) and apply r later as a
            scale in the PSUM->SBUF copy."""
            scr = drp.tile([GS, nsets * K], F32, tag="scr" + tag)
            nc.sync.dma_start(scr, lg_sb)
            lgb = attp.tile([128, GS, nsets, K], F32, tag="lgb" + tag)
            nc.sync.dma_start(lgb, _ap(scr, 0, [[0, 128], [1, GS * nsets * K]]))
            e = attp.tile([128, GS, nsets, K], F32, tag="e" + tag)
            nc.scalar.activation(e, lgb, AFT.Exp, scale=1.0 / TEMP)
            esum = attp.tile([128, GS, nsets], F32, tag="es" + tag)
            nc.vector.reduce_sum(esum, e, axis=mybir.AxisListType.X)
            r = attp.tile([128, GS, nsets], F32, tag="r" + tag)
            nc.vector.reciprocal(r, esum)
            return e, r

        def build_aw(wT, ncs, att_sc, tag):
            """aw[p, c, o] = sum_k att_k * wT[p, c, k, o]; att_sc(k)->[128,1] AP"""
            aw = awp.tile([128, ncs, C1], CDT, tag=tag)
            nc.vector.tensor_scalar_mul(aw, wT[:, :, 0, :], att_sc(0))
            for k in range(1, K):
                nc.vector.scalar_tensor_tensor(
                    aw, wT[:, :, k, :], att_sc(k), aw, op0=OP.mult, op1=OP.add
                )
            return aw

        def build_ab_batched(bT, att, set_idx, out_ap):
            """out[:, s] = sum_k att[:, s, set_idx, k] * bT[:, k] for all samples
            in one 4-op chain (free dim = samples, scalar = bT column)."""
            nc.vector.tensor_scalar_mul(out_ap, att[:, :, set_idx, 0], bT[:, 0:1])
            for k in range(1, K):
                nc.vector.scalar_tensor_tensor(
                    out_ap, att[:, :, set_idx, k], bT[:, k : k + 1], out_ap,
                    op0=OP.mult, op1=OP.add,
                )

        ab_u2 = stats.tile([128, BL], F32, tag="ab_u2")
        ab_u1 = stats.tile([128, BL], F32, tag="ab_u1")
        ab_dl = stats.tile([128, BL], F32, tag="ab_dl")
        xus = stats.tile([128, BL], F32, tag="xus")
        X2U = [None] * BL
        L = [None] * BL

        for g in range(NG):
            sl = slice(g * GS, (g + 1) * GS)
            ss = list(range(g * GS, (g + 1) * GS))

            # ---- u2 + u1 attention for this group ----
            if ABLATE == "noatt":
                e01 = attp.tile([128, GS, 2, K], F32, tag=f"atta{g}")
                nc.vector.memset(e01, 0.25)
                r01 = attp.tile([128, GS, 2], F32, tag=f"attar{g}")
                nc.vector.memset(r01, 1.0)
            else:
                nc.vector.tensor_scalar_mul(avg_x2[:, sl], sum_x2[:, sl], 1.0 / HW2)
                nc.vector.tensor_scalar_mul(avg_x3[:, :, sl], avg_x3[:, :, sl], 1.0 / HW3)
                lg_sb = attp.tile([GS, 2 * K], F32, tag=f"lgsb{g}")
                h_u2 = att_mlp(p_u2f1, 1, 2, avg_x2[:, sl], f"hu2{g}")
                lg_u2 = smps.tile([GS, K], F32, tag="sm")
                att_fc2(h_u2, p_u2f2, 2, lg_u2)
                nc.vector.tensor_tensor(lg_sb[:, 0:K], lg_u2, p_f2b[0:GS, 0:K], op=OP.add)
                h_u1 = att_mlp(p_u1f1, 2, 3, avg_x3[:, :, sl], f"hu1{g}")
                lg_u1 = smps.tile([GS, K], F32, tag="sm")
                att_fc2(h_u1, p_u1f2, 3, lg_u1)
                nc.vector.tensor_tensor(lg_sb[:, K : 2 * K], lg_u1,
                                        p_f2b[0:GS, K : 2 * K], op=OP.add)
                e01, r01 = exp_bcast(lg_sb, 2, f"a{g}")  # [128, j, set, K]

            # ---- aw/ab + u2/u1 convs per sample ----
            build_ab_batched(p_u2b, e01, 0, ab_u2[:, sl])
            nc.vector.tensor_tensor(ab_u2[:, sl], ab_u2[:, sl], r01[:, :, 0], op=OP.mult)
            build_ab_batched(p_u1b, e01, 1, ab_u1[:, sl])
            nc.vector.tensor_tensor(ab_u1[:, sl], ab_u1[:, sl], r01[:, :, 1], op=OP.mult)
            for j, s in enumerate(ss):
                a2 = build_aw(p_u2w, 1, lambda k: e01[:, j, 0, k : k + 1], f"aw2_{s}")
                a1 = build_aw(p_u1w, 2, lambda k: e01[:, j, 1, k : k + 1], f"aw1_{s}")

                xu = keep.tile([128, HW2], CDT, tag=f"x2u{s}")
                for jj in range(4):
                    ps = bigps.tile([128, 1024], F32, tag="ps")
                    for half in range(2):
                        nc.tensor.matmul(
                            ps[:, 512 * half : 512 * (half + 1)], a2,
                            X2[s][:, 1024 * jj + 512 * half : 1024 * jj + 512 * (half + 1)],
                            start=True, stop=True,
                        )
                    nc.scalar.activation(
                        xu[:, 1024 * jj : 1024 * (jj + 1)], ps, AFT.Identity,
                        bias=ab_u2[:, s : s + 1], scale=r01[:, j, 0:1],
                        accum_out=xu_part[:, jj, s : s + 1],
                    )
                X2U[s] = xu
                if not ABLATE:
                    nc.vector.reduce_max(V[:, 0, 1, s : s + 1], xu, axis=mybir.AxisListType.X)

                lt = keep.tile([128, HW3], CDT, tag=f"l{s}")
                psl = bigps.tile([128, 1024], F32, tag="ps")
                for half in range(2):
                    for c in range(2):
                        nc.tensor.matmul(
                            psl[:, 512 * half : 512 * (half + 1)], a1[:, c, :],
                            X3[s][:, c, 512 * half : 512 * (half + 1)],
                            start=(c == 0), stop=(c == 1),
                        )
                nc.scalar.activation(
                    lt, psl, AFT.Identity,
                    bias=ab_u1[:, s : s + 1], scale=r01[:, j, 1:2],
                    accum_out=lsum[:, s : s + 1],
                )
                L[s] = lt
                if not ABLATE:
                    nc.vector.reduce_max(V[:, 1, 1, s : s + 1], lt, axis=mybir.AxisListType.X)

            # ---- CA + dl1 attention for this group ----
            if ABLATE == "noatt":
                nc.vector.memset(mask[:, :, sl], 0.5)
                e_dl = attp.tile([128, GS, 1, K], F32, tag=f"attb{g}")
                nc.vector.memset(e_dl, 0.25)
                r_dl = attp.tile([128, GS, 1], F32, tag=f"attbr{g}")
                nc.vector.memset(r_dl, 1.0)
            else:
                nc.vector.reduce_sum(
                    xus[:, sl], xu_part.transpose([0, 2, 1])[:, sl, :],
                    axis=mybir.AxisListType.X,
                )
                nc.vector.tensor_scalar_mul(V[:, 0, 0, sl], xus[:, sl], 1.0 / HW2)
                nc.vector.tensor_scalar_mul(V[:, 1, 0, sl], lsum[:, sl], 1.0 / HW3)

                h1p = smps.tile([128, 2, GS], F32, tag="sm")
                for c in range(2):
                    nc.tensor.matmul(
                        h1p, p_cw1[:, c, :], V[:, c, :, sl],
                        start=(c == 0), stop=(c == 1),
                    )
                h1 = attp.tile([128, 2, GS], F32, tag=f"h1{g}")
                nc.scalar.activation(h1, h1p, AFT.Relu, bias=p_cb1)
                # fus(avg)+fus(max) = w2 @ (h1_avg + h1_max): sum before mm,
                # then exp reads the mm PSUM directly with bias = -2*ca_b2.
                h1s = attp.tile([128, GS], F32, tag=f"h1s{g}")
                nc.vector.tensor_tensor(h1s, h1[:, 0, :], h1[:, 1, :], op=OP.add)
                z0 = smps.tile([128, GS], F32, tag="sm")
                z1 = smps.tile([128, GS], F32, tag="sm")
                nc.tensor.matmul(z0, p_cw2[:, 0:128], h1s, start=True, stop=True)
                nc.tensor.matmul(z1, p_cw2[:, 128:256], h1s, start=True, stop=True)
                emk = attp.tile([128, 2, GS], F32, tag=f"emk{g}")
                nc.scalar.activation(emk[:, 0, :], z0, AFT.Exp, scale=-1.0, bias=p_cb2[:, 0:1])
                nc.scalar.activation(emk[:, 1, :], z1, AFT.Exp, scale=-1.0, bias=p_cb2[:, 1:2])
                nc.vector.tensor_scalar_add(emk, emk, 1.0)
                nc.vector.reciprocal(mask[:, :, sl], emk)

                nc.vector.tensor_tensor(pooled_dl[:, 0, sl], V[:, 0, 0, sl],
                                        mask[:, 0, sl], op=OP.mult)
                nc.vector.tensor_tensor(pooled_dl[:, 1, sl], V[:, 1, 0, sl],
                                        mask[:, 1, sl], op=OP.mult)
                h_dl = att_mlp(p_dlf1, 2, 3, pooled_dl[:, :, sl], f"hdl{g}")
                lg_dl = smps.tile([GS, K], F32, tag="sm")
                att_fc2(h_dl, p_dlf2, 3, lg_dl)
                lg_sb2 = attp.tile([GS, K], F32, tag=f"lgsb2{g}")
                nc.vector.tensor_tensor(lg_sb2, lg_dl, p_f2b[0:GS, 2 * K : 3 * K], op=OP.add)
                e_dl, r_dl = exp_bcast(lg_sb2, 1, f"b{g}")  # [128, j, 1, K]

            # ---- dl1 conv per sample ----
            build_ab_batched(p_dlb, e_dl, 0, ab_dl[:, sl])
            nc.vector.tensor_tensor(ab_dl[:, sl], ab_dl[:, sl], r_dl[:, :, 0], op=OP.mult)
            for j, s in enumerate(ss):
                matt = attp.tile([128, 2, K], F32, tag=f"matt{g}")
                for c in range(2):
                    nc.vector.tensor_scalar_mul(
                        matt[:, c, :], e_dl[:, j, 0, :], mask[:, c, s : s + 1]
                    )
                awd = awp.tile([128, 2, C1], CDT, tag=f"awd_{s}")
                for c in range(2):
                    eng = nc.gpsimd if (AWD_GP and c == 1) else nc.vector
                    eng.tensor_scalar_mul(
                        awd[:, c, :], p_dlw[:, c, 0, :], matt[:, c, 0:1]
                    )
                    for k in range(1, K):
                        eng.scalar_tensor_tensor(
                            awd[:, c, :], p_dlw[:, c, k, :], matt[:, c, k : k + 1],
                            awd[:, c, :], op0=OP.mult, op1=OP.add,
                        )

                # out in GROUPED spatial layout: col = h'*64 + parity*32 + w
                # (w' = 2w + parity); host un-interleaves.
                ot = outp.tile([128, HW2], F32, tag="out")
                for jj in range(4):
                    ps = bigps.tile([128, 1024], F32, tag="ps")
                    for half in range(2):
                        bank = ps[:, 512 * half : 512 * (half + 1)]
                        t = 2 * jj + half  # 512-block: h' rows 8t..8t+7
                        rhs0 = _ap(
                            X2U[s], 512 * t,
                            [list(X2U[s].ap[0]), [64, 8], [1, 2], [2, 32]],
                        )
                        nc.tensor.matmul(bank, awd[:, 0, :], rhs0, start=True, stop=False)
                        rhs1 = _ap(
                            L[s], 4 * t * 32,
                            [list(L[s].ap[0]), [32, 4], [0, 4], [1, 32]],
                        )
                        nc.tensor.matmul(bank, awd[:, 1, :], rhs1, start=False, stop=True)
                    nc.scalar.activation(
                        ot[:, 1024 * jj : 1024 * jj + 512], ps[:, 0:512],
                        AFT.Identity, bias=ab_dl[:, s : s + 1], scale=r_dl[:, j, 0:1],
                    )
                    nc.vector.tensor_scalar(
                        ot[:, 1024 * jj + 512 : 1024 * (jj + 1)],
                        ps[:, 512:1024], r_dl[:, j, 0:1], ab_dl[:, s : s + 1],
                        op0=OP.mult, op1=OP.add,
                    )
                    nc.sync.dma_start(
                        T["y"].ap()[s, :, 1024 * jj : 1024 * (jj + 1)],
                        ot[:, 1024 * jj : 1024 * (jj + 1)],
                    )


def _prep_params(i):
    """Host-side param preprocessing -> dict of DRAM arrays (shared by cores)."""
    f32 = np.float32
    bf = ml_dtypes.bfloat16

    def wT(w):  # [K, Co, Ci] -> [Ci//128, 128, K, Co]
        ci = w.shape[2]
        return np.ascontiguousarray(
            w.transpose(2, 0, 1).reshape(ci // 128, 128, K, w.shape[1])
        ).astype(bf)

    def fc1T(w, hid_pad):  # [Hid, C] -> [C//128, 128, hid_pad]
        c = w.shape[1]
        out = np.zeros((c // 128, 128, hid_pad), f32)
        out[:, :, : w.shape[0]] = w.T.reshape(c // 128, 128, w.shape[0])
        return out

    def fc2T(w, nh):  # [K, Hid] -> [nh, 128, K]
        out = np.zeros((nh, 128, K), f32)
        out.reshape(nh * 128, K)[: w.shape[1], :] = w.T
        return out

    fc2b = np.zeros((4, 3 * K), f32)
    fc2b[:, 0:K] = i["u2_fc2_b"][None, :]
    fc2b[:, K : 2 * K] = i["u1_fc2_b"][None, :]
    fc2b[:, 2 * K : 3 * K] = i["dl1_fc2_b"][None, :]

    return {
        "u2_wT": wT(i["u2_w"]),
        "u1_wT": wT(i["u1_w"]),
        "dl1_wT": wT(i["dl1_w"]),
        "u2_bT": np.ascontiguousarray(i["u2_b"].T).astype(f32),
        "u1_bT": np.ascontiguousarray(i["u1_b"].T).astype(f32),
        "dl1_bT": np.ascontiguousarray(i["dl1_b"].T).astype(f32),
        "u2_fc1T": fc1T(i["u2_fc1_w"], 256),
        "u1_fc1T": fc1T(i["u1_fc1_w"], 384),
        "dl1_fc1T": fc1T(i["dl1_fc1_w"], 384),
        "u2_fc2T": fc2T(i["u2_fc2_w"], 2),
        "u1_fc2T": fc2T(i["u1_fc2_w"], 3),
        "dl1_fc2T": fc2T(i["dl1_fc2_w"], 3),
        "fc2_b": fc2b,
        "ca_w1T": np.ascontiguousarray(i["ca_w1"].T.reshape(2, 128, C1)).astype(f32),
        "ca_w2T": np.ascontiguousarray(i["ca_w2"].T).astype(f32),
        "ca_b1": np.ascontiguousarray(i["ca_b1"][:, None]).astype(f32),
        # fus(avg)+fus(max) each add ca_b2 -> 2*ca_b2; negated because it is
        # applied as the bias of exp(-z - 2*ca_b2) in the sigmoid
        "ca_b2": np.ascontiguousarray(-2.0 * i["ca_b2"].reshape(2, 128)).astype(f32),
    }


def make_in_maps(**inputs):
    bf = ml_dtypes.bfloat16
    params = _prep_params(inputs)
    x2 = np.asarray(inputs["x2"]).reshape(B, C1, HW2).astype(bf)
    x3 = np.asarray(inputs["x3"]).reshape(B, 2, 128, HW3).astype(bf)
    in_maps = []
    for c in range(N_CORES):
        m = dict(params)
        m["x2"] = np.ascontiguousarray(x2[c * BL : (c + 1) * BL])
        m["x3"] = np.ascontiguousarray(x3[c * BL : (c + 1) * BL])
        in_maps.append(m)
    return in_maps


_NC_CACHE = None


def get_nc():
    global _NC_CACHE
    if _NC_CACHE is None:
        _NC_CACHE = build_nc()
    return _NC_CACHE


def unpack_out(y_cores):
    """y per core [BL, C1, HW2] in grouped layout (col = h'*64 + p*32 + w,
    w' = 2w + p) -> full [B, C1, 64, 64]."""
    out = np.concatenate(y_cores, axis=0).reshape(B, C1, 64, 2, 32)
    return np.ascontiguousarray(out.transpose(0, 1, 2, 4, 3).reshape(B, C1, 64, 64))


def kernel(**inputs):
    nc = get_nc()
    in_maps = make_in_maps(**inputs)
    res = run_bass_kernel_spmd(nc, in_maps, core_ids=list(range(N_CORES)))
    return unpack_out([res.results[c]["y"] for c in range(N_CORES)]).astype(np.float32)


# revision 43
# speedup vs baseline: 1.0075x; 1.0075x over previous
"""Trainium2 Bass kernel for nn_BM2_15822659518813 (dense_cnn).

Pipeline per sample (B=32 sharded 4-per-core across 8 cores):
  x2u = DynConv1x1(x2; u2)              # 128->128 on 64x64
  l   = DynConv1x1(x3; u1)              # 256->128 on 32x32
  lr  = cat(x2u, upsample2x(l))         # 256ch, 64x64   (never materialized)
  b   = CA(lr)                          # channel mask, folded into dl1 weights
  out = DynConv1x1(b; dl1)              # 256->128 on 64x64

Key tricks:
  - per-sample dynamic 1x1 kernels (aw) built on DVE via scalar_tensor_tensor
  - CA mask folded into dl1 aw rows (no full-tensor multiply)
  - nearest 2x upsample via 0-step replicated matmul rhs APs (free on PE)
  - avg-pools fused into PSUM->SBUF copies via ACT accum_out
  - sigmoid via exp (stays in one ACT table set)
  - softmax partition-broadcast via tiny DRAM bounce
"""

import sys

if "/opt/trn_rl_repo" not in sys.path:
    sys.path.insert(0, "/opt/trn_rl_repo")

import numpy as np
import ml_dtypes

import concourse.bacc as bacc
import concourse.bass as bass
import concourse.tile as tile
import concourse.mybir as mybir
from concourse.bass_utils import run_bass_kernel_spmd

F32 = mybir.dt.float32
BF16 = mybir.dt.bfloat16
AFT = mybir.ActivationFunctionType
OP = mybir.AluOpType

N_CORES = 8
B = 32
BL = B // N_CORES          # 4 samples per core
C1 = 128
C2 = 256
K = 4
HW2 = 64 * 64              # 4096
HW3 = 32 * 32              # 1024
TEMP = 34.0

CDT = BF16                 # compute dtype for matmul operands
DEBUG = False              # emit debug DRAM outputs (sim only)
REPEAT = 1                 # >1: wrap body in a HW loop (timing builds only)
ABLATE = ""                # "", "dmaonly", "noatt" (timing diagnostics)
AWD_GP = False             # build dl1 aw chunk 1 on GPSIMD (parallel to DVE)


def _ap(t, offset_extra, dims):
    return bass.AP(tensor=t.tensor, offset=t.offset + offset_extra, ap=dims)


def build_nc():
    nc = bacc.Bacc("TRN2", target_bir_lowering=False, debug=False)

    # ---------- DRAM I/O ----------
    x2 = nc.dram_tensor("x2", [BL, C1, HW2], CDT, kind="ExternalInput")
    x3 = nc.dram_tensor("x3", [BL, 2, 128, HW3], CDT, kind="ExternalInput")
    y = nc.dram_tensor("y", [BL, C1, HW2], F32, kind="ExternalOutput")

    # params (host pre-transposed; see kernel() for layouts)
    u2_wT = nc.dram_tensor("u2_wT", [1, 128, K, C1], CDT, kind="ExternalInput")
    u1_wT = nc.dram_tensor("u1_wT", [2, 128, K, C1], CDT, kind="ExternalInput")
    dl1_wT = nc.dram_tensor("dl1_wT", [2, 128, K, C1], CDT, kind="ExternalInput")
    u2_bT = nc.dram_tensor("u2_bT", [C1, K], F32, kind="ExternalInput")
    u1_bT = nc.dram_tensor("u1_bT", [C1, K], F32, kind="ExternalInput")
    dl1_bT = nc.dram_tensor("dl1_bT", [C1, K], F32, kind="ExternalInput")
    # fc1 lhsT: [c_chunks, 128, hid_pad]; fc2 rhs: [hid_chunks, 128, K]
    u2_fc1T = nc.dram_tensor("u2_fc1T", [1, 128, 256], F32, kind="ExternalInput")
    u1_fc1T = nc.dram_tensor("u1_fc1T", [2, 128, 384], F32, kind="ExternalInput")
    dl1_fc1T = nc.dram_tensor("dl1_fc1T", [2, 128, 384], F32, kind="ExternalInput")
    u2_fc2T = nc.dram_tensor("u2_fc2T", [2, 128, K], F32, kind="ExternalInput")
    u1_fc2T = nc.dram_tensor("u1_fc2T", [3, 128, K], F32, kind="ExternalInput")
    dl1_fc2T = nc.dram_tensor("dl1_fc2T", [3, 128, K], F32, kind="ExternalInput")
    fc2_b = nc.dram_tensor("fc2_b", [4, 3 * K], F32, kind="ExternalInput")  # [b<=4, (set,k)]
    ca_w1T = nc.dram_tensor("ca_w1T", [2, 128, C1], F32, kind="ExternalInput")
    ca_w2T = nc.dram_tensor("ca_w2T", [128, C2], F32, kind="ExternalInput")
    ca_b1 = nc.dram_tensor("ca_b1", [C1, 1], F32, kind="ExternalInput")
    ca_b2 = nc.dram_tensor("ca_b2", [2, 128], F32, kind="ExternalInput")

    dbg = {}
    if DEBUG:
        dbg = {
            "d_avgx2": nc.dram_tensor("d_avgx2", [128, BL], F32, kind="ExternalOutput"),
            "d_att01": nc.dram_tensor("d_att01", [128, BL, 2, K], F32, kind="ExternalOutput"),
            "d_mask": nc.dram_tensor("d_mask", [128, 2, BL], F32, kind="ExternalOutput"),
            "d_V": nc.dram_tensor("d_V", [128, 2, 2, BL], F32, kind="ExternalOutput"),
            "d_x2u0": nc.dram_tensor("d_x2u0", [128, HW2], CDT, kind="ExternalOutput"),
            "d_l0": nc.dram_tensor("d_l0", [128, HW3], CDT, kind="ExternalOutput"),
            "d_aw2_0": nc.dram_tensor("d_aw2_0", [128, C1], CDT, kind="ExternalOutput"),
            "d_awd0": nc.dram_tensor("d_awd0", [128, 2, C1], CDT, kind="ExternalOutput"),
            "d_attdl": nc.dram_tensor("d_attdl", [128, BL, 1, K], F32, kind="ExternalOutput"),
            "d_abdl": nc.dram_tensor("d_abdl", [128, BL], F32, kind="ExternalOutput"),
        }
    with tile.TileContext(nc) as tc:
        _emit(nc, tc, {**locals(), **dbg})
    nc.compile()
    return nc


def _emit(nc, tc, T):
    import contextlib

    ctx = contextlib.ExitStack()
    with ctx:
        if REPEAT > 1:
            ctx.enter_context(
                tc.For_i(0, REPEAT, 1, hint_engines=tuple(mybir.ALL_ENGINES))
            )
        par = ctx.enter_context(tc.tile_pool(name="par", bufs=1))
        stats = ctx.enter_context(tc.tile_pool(name="stats", bufs=1))
        xin = ctx.enter_context(tc.tile_pool(name="xin", bufs=4))
        x3in = ctx.enter_context(tc.tile_pool(name="x3in", bufs=4))
        keep = ctx.enter_context(tc.tile_pool(name="keep", bufs=1))
        outp = ctx.enter_context(tc.tile_pool(name="outp", bufs=3))
        awp = ctx.enter_context(tc.tile_pool(name="awp", bufs=1))
        attp = ctx.enter_context(tc.tile_pool(name="attp", bufs=2))
        bigps = ctx.enter_context(tc.tile_pool(name="bigps", bufs=2, space="PSUM"))
        smps = ctx.enter_context(tc.tile_pool(name="smps", bufs=4, space="PSUM"))
        drp = ctx.enter_context(tc.tile_pool(name="drp", bufs=2, space="DRAM"))

        # ---------- load params ----------
        def ld(dram, shape, transpose=None, dtype=None):
            t = par.tile(shape, dtype or dram.ap().dtype, tag=dram.ap().name)
            src = dram.ap()
            if transpose:
                src = src.transpose(transpose)
            nc.sync.dma_start(t, src)
            return t

        p_u2w = ld(T["u2_wT"], [128, 1, K, C1], [1, 0, 2, 3])
        p_u1w = ld(T["u1_wT"], [128, 2, K, C1], [1, 0, 2, 3])
        p_dlw = ld(T["dl1_wT"], [128, 2, K, C1], [1, 0, 2, 3])
        p_u2b = ld(T["u2_bT"], [C1, K])
        p_u1b = ld(T["u1_bT"], [C1, K])
        p_dlb = ld(T["dl1_bT"], [C1, K])
        p_u2f1 = ld(T["u2_fc1T"], [128, 1, 256], [1, 0, 2])
        p_u1f1 = ld(T["u1_fc1T"], [128, 2, 384], [1, 0, 2])
        p_dlf1 = ld(T["dl1_fc1T"], [128, 2, 384], [1, 0, 2])
        p_u2f2 = ld(T["u2_fc2T"], [128, 2, K], [1, 0, 2])
        p_u1f2 = ld(T["u1_fc2T"], [128, 3, K], [1, 0, 2])
        p_dlf2 = ld(T["dl1_fc2T"], [128, 3, K], [1, 0, 2])
        p_f2b = ld(T["fc2_b"], [4, 3 * K])
        p_cw1 = ld(T["ca_w1T"], [128, 2, C1], [1, 0, 2])
        p_cw2 = ld(T["ca_w2T"], [128, C2])
        p_cb1 = ld(T["ca_b1"], [C1, 1])
        p_cb2 = ld(T["ca_b2"], [128, 2], [1, 0])

        # ---------- stats tiles ----------
        sum_x2 = stats.tile([128, BL], F32, tag="sum_x2")      # raw sums of x2
        avg_x3 = stats.tile([128, 2, BL], F32, tag="avg_x3")
        xu_part = stats.tile([128, 4, BL], F32, tag="xu_part")  # x2u partial sums
        V = stats.tile([128, 2, 2, BL], F32, tag="V")           # [c-chunk, avg/max, s]
        lsum = stats.tile([128, BL], F32, tag="lsum")
        avg_x2 = stats.tile([128, BL], F32, tag="avg_x2")
        mask = stats.tile([128, 2, BL], F32, tag="mask")
        pooled_dl = stats.tile([128, 2, BL], F32, tag="pooled_dl")

        # ---------- per-sample input DMA + input pooling ----------
        X2 = []
        X3 = []
        for s in range(BL):
            t2 = xin.tile([128, HW2], CDT, tag="x2")
            nc.sync.dma_start(t2, T["x2"].ap()[s, :, :])
            t3 = x3in.tile([128, 2, HW3], CDT, tag="x3")
            nc.sync.dma_start(t3, T["x3"].ap()[s, :, :, :].transpose([1, 0, 2]))
            X2.append(t2)
            X3.append(t3)
            if ABLATE:
                continue
            nc.vector.reduce_sum(sum_x2[:, s : s + 1], t2, axis=mybir.AxisListType.X)
            nc.vector.reduce_sum(avg_x3[:, :, s], t3, axis=mybir.AxisListType.X)
        if ABLATE == "dmaonly":
            for s in range(BL):
                v = X2[s].bitcast(F32)
                nc.sync.dma_start(T["y"].ap()[s, :, 0 : HW2 // 2], v)
                nc.sync.dma_start(T["y"].ap()[s, :, HW2 // 2 : HW2], v)
            return
        if not ABLATE:
            nc.vector.tensor_scalar_mul(avg_x2, sum_x2, 1.0 / HW2)
            nc.vector.tensor_scalar_mul(avg_x3, avg_x3, 1.0 / HW3)

        # ---------- helpers (group-parameterized) ----------
        GS = 4                      # samples per group
        NG = BL // GS

        def att_mlp(fc1T, ncs, nh, pooled, tag):
            """fc1T: [128, ncs, hidpad]; pooled sliced to GS columns."""
            h = attp.tile([128, nh, GS], F32, tag=tag)
            for m in range(nh):
                hp = smps.tile([128, GS], F32, tag="sm")
                for c in range(ncs):
                    rhs = pooled[:, c, :] if ncs > 1 else pooled
                    nc.tensor.matmul(
                        hp, fc1T[:, c, 128 * m : 128 * (m + 1)], rhs,
                        start=(c == 0), stop=(c == ncs - 1),
                    )
                nc.scalar.activation(h[:, m, :], hp, AFT.Relu)
            return h

        def att_fc2(h, fc2T, nh, lg_ps):
            for m in range(nh):
                nc.tensor.matmul(
                    lg_ps, h[:, m, :], fc2T[:, m, :],
                    start=(m == 0), stop=(m == nh - 1),
                )

        def exp_bcast(lg_sb, nsets, tag):
            """lg_sb: [GS, nsets*K] logits+bias -> (e, r): unnormalized
            exp [128, GS, nsets, K] and 1/sum [128, GS, nsets]. Consumers
            build aw/ab from e (no wait on the sum) and apply r later as a
            scale in the PSUM->SBUF copy."""
            scr = drp.tile([GS, nsets * K], F32, tag="scr" + tag)
            nc.sync.dma_start(scr, lg_sb)
            lgb = attp.tile([128, GS, nsets, K], F32, tag="lgb" + tag)
            nc.sync.dma_start(lgb, _ap(scr, 0, [[0, 128], [1, GS * nsets * K]]))
            e = attp.tile([128, GS, nsets, K], F32, tag="e" + tag)
            nc.scalar.activation(e, lgb, AFT.Exp, scale=1.0 / TEMP)
            esum = attp.tile([128, GS, nsets], F32, tag="es" + tag)
            nc.vector.reduce_sum(esum, e, axis=mybir.AxisListType.X)
            r = attp.tile([128, GS, nsets], F32, tag="r" + tag)
            nc.vector.reciprocal(r, esum)
            return e, r

        def build_aw(wT, ncs, att_sc, tag):
            """aw[p, c, o] = sum_k att_k * wT[p, c, k, o]; att_sc(k)->[128,1] AP"""
            aw = awp.tile([128, ncs, C1], CDT, tag=tag)
            nc.vector.tensor_scalar_mul(aw, wT[:, :, 0, :], att_sc(0))
            for k in range(1, K):
                nc.vector.scalar_tensor_tensor(
                    aw, wT[:, :, k, :], att_sc(k), aw, op0=OP.mult, op1=OP.add
                )
            return aw

        def build_ab_batched(bT, att, set_idx, out_ap):
            """out[:, s] = sum_k att[:, s, set_idx, k] * bT[:, k] for all samples
            in one 4-op chain (free dim = samples, scalar = bT column)."""
            nc.vector.tensor_scalar_mul(out_ap, att[:, :, set_idx, 0], bT[:, 0:1])
            for k in range(1, K):
                nc.vector.scalar_tensor_tensor(
                    out_ap, att[:, :, set_idx, k], bT[:, k : k + 1], out_ap,
                    op0=OP.mult, op1=OP.add,
                )

        ab_u2 = stats.tile([128, BL], F32, tag="ab_u2")
        ab_u1 = stats.tile([128, BL], F32, tag="ab_u1")
        ab_dl = stats.tile([128, BL], F32, tag="ab_dl")
        xus = stats.tile([128, BL], F32, tag="xus")
        X2U = [None] * BL
        L = [None] * BL

        for g in range(NG):
            sl = slice(g * GS, (g + 1) * GS)
            ss = list(range(g * GS, (g + 1) * GS))

            # ---- u2 + u1 attention for this group ----
            if ABLATE == "noatt":
                e01 = attp.tile([128, GS, 2, K], F32, tag=f"atta{g}")
                nc.vector.memset(e01, 0.25)
                r01 = attp.tile([128, GS, 2], F32, tag=f"attar{g}")
                nc.vector.memset(r01, 1.0)
            else:
                nc.vector.tensor_scalar_mul(avg_x2[:, sl], sum_x2[:, sl], 1.0 / HW2)
                nc.vector.tensor_scalar_mul(avg_x3[:, :, sl], avg_x3[:, :, sl], 1.0 / HW3)
                lg_sb = attp.tile([GS, 2 * K], F32, tag=f"lgsb{g}")
                h_u2 = att_mlp(p_u2f1, 1, 2, avg_x2[:, sl], f"hu2{g}")
                lg_u2 = smps.tile([GS, K], F32, tag="sm")
                att_fc2(h_u2, p_u2f2, 2, lg_u2)
                nc.vector.tensor_tensor(lg_sb[:, 0:K], lg_u2, p_f2b[0:GS, 0:K], op=OP.add)
                h_u1 = att_mlp(p_u1f1, 2, 3, avg_x3[:, :, sl], f"hu1{g}")
                lg_u1 = smps.tile([GS, K], F32, tag="sm")
                att_fc2(h_u1, p_u1f2, 3, lg_u1)
                nc.vector.tensor_tensor(lg_sb[:, K : 2 * K], lg_u1,
                                        p_f2b[0:GS, K : 2 * K], op=OP.add)
                e01, r01 = exp_bcast(lg_sb, 2, f"a{g}")  # [128, j, set, K]

            # ---- aw/ab + u2/u1 convs per sample ----
            build_ab_batched(p_u2b, e01, 0, ab_u2[:, sl])
            nc.vector.tensor_tensor(ab_u2[:, sl], ab_u2[:, sl], r01[:, :, 0], op=OP.mult)
            build_ab_batched(p_u1b, e01, 1, ab_u1[:, sl])
            nc.vector.tensor_tensor(ab_u1[:, sl], ab_u1[:, sl], r01[:, :, 1], op=OP.mult)
            for j, s in enumerate(ss):
                a2 = build_aw(p_u2w, 1, lambda k: e01[:, j, 0, k : k + 1], f"aw2_{s}")
                a1 = build_aw(p_u1w, 2, lambda k: e01[:, j, 1, k : k + 1], f"aw1_{s}")

                xu = keep.tile([128, HW2], CDT, tag=f"x2u{s}")
                for jj in range(4):
                    ps = bigps.tile([128, 1024], F32, tag="ps")
                    for half in range(2):
                        nc.tensor.matmul(
                            ps[:, 512 * half : 512 * (half + 1)], a2,
                            X2[s][:, 1024 * jj + 512 * half : 1024 * jj + 512 * (half + 1)],
                            start=True, stop=True,
                        )
                    nc.scalar.activation(
                        xu[:, 1024 * jj : 1024 * (jj + 1)], ps, AFT.Identity,
                        bias=ab_u2[:, s : s + 1], scale=r01[:, j, 0:1],
                        accum_out=xu_part[:, jj, s : s + 1],
                    )
                X2U[s] = xu
                if not ABLATE:
                    nc.vector.reduce_max(V[:, 0, 1, s : s + 1], xu, axis=mybir.AxisListType.X)

                lt = keep.tile([128, HW3], CDT, tag=f"l{s}")
                psl = bigps.tile([128, 1024], F32, tag="ps")
                for half in range(2):
                    for c in range(2):
                        nc.tensor.matmul(
                            psl[:, 512 * half : 512 * (half + 1)], a1[:, c, :],
                            X3[s][:, c, 512 * half : 512 * (half + 1)],
                            start=(c == 0), stop=(c == 1),
                        )
                nc.scalar.activation(
                    lt, psl, AFT.Identity,
                    bias=ab_u1[:, s : s + 1], scale=r01[:, j, 1:2],
                    accum_out=lsum[:, s : s + 1],
                )
                L[s] = lt
                if not ABLATE:
                    nc.vector.reduce_max(V[:, 1, 1, s : s + 1], lt, axis=mybir.AxisListType.X)

            # ---- CA + dl1 attention for this group ----
            if ABLATE == "noatt":
                nc.vector.memset(mask[:, :, sl], 0.5)
                e_dl = attp.tile([128, GS, 1, K], F32, tag=f"attb{g}")
                nc.vector.memset(e_dl, 0.25)
                r_dl = attp.tile([128, GS, 1], F32, tag=f"attbr{g}")
                nc.vector.memset(r_dl, 1.0)
            else:
                nc.vector.reduce_sum(
                    xus[:, sl], xu_part.transpose([0, 2, 1])[:, sl, :],
                    axis=mybir.AxisListType.X,
                )
                nc.vector.tensor_scalar_mul(V[:, 0, 0, sl], xus[:, sl], 1.0 / HW2)
                nc.vector.tensor_scalar_mul(V[:, 1, 0, sl], lsum[:, sl], 1.0 / HW3)

                h1p = smps.tile([128, 2, GS], F32, tag="sm")
                for c in range(2):
                    nc.tensor.matmul(
                        h1p, p_cw1[:, c, :], V[:, c, :, sl],
                        start=(c == 0), stop=(c == 1),
                    )
                h1 = attp.tile([128, 2, GS], F32, tag=f"h1{g}")
                nc.scalar.activation(h1, h1p, AFT.Relu, bias=p_cb1)
                # fus(avg)+fus(max) = w2 @ (h1_avg + h1_max): sum before mm,
                # then exp reads the mm PSUM directly with bias = -2*ca_b2.
                h1s = attp.tile([128, GS], F32, tag=f"h1s{g}")
                nc.vector.tensor_tensor(h1s, h1[:, 0, :], h1[:, 1, :], op=OP.add)
                z0 = smps.tile([128, GS], F32, tag="sm")
                z1 = smps.tile([128, GS], F32, tag="sm")
                nc.tensor.matmul(z0, p_cw2[:, 0:128], h1s, start=True, stop=True)
                nc.tensor.matmul(z1, p_cw2[:, 128:256], h1s, start=True, stop=True)
                emk = attp.tile([128, 2, GS], F32, tag=f"emk{g}")
                nc.scalar.activation(emk[:, 0, :], z0, AFT.Exp, scale=-1.0, bias=p_cb2[:, 0:1])
                nc.scalar.activation(emk[:, 1, :], z1, AFT.Exp, scale=-1.0, bias=p_cb2[:, 1:2])
                nc.vector.tensor_scalar_add(emk, emk, 1.0)
                nc.vector.reciprocal(mask[:, :, sl], emk)

                nc.vector.tensor_tensor(pooled_dl[:, 0, sl], V[:, 0, 0, sl],
                                        mask[:, 0, sl], op=OP.mult)
                nc.vector.tensor_tensor(pooled_dl[:, 1, sl], V[:, 1, 0, sl],
                                        mask[:, 1, sl], op=OP.mult)
                h_dl = att_mlp(p_dlf1, 2, 3, pooled_dl[:, :, sl], f"hdl{g}")
                lg_dl = smps.tile([GS, K], F32, tag="sm")
                att_fc2(h_dl, p_dlf2, 3, lg_dl)
                lg_sb2 = attp.tile([GS, K], F32, tag=f"lgsb2{g}")
                nc.vector.tensor_tensor(lg_sb2, lg_dl, p_f2b[0:GS, 2 * K : 3 * K], op=OP.add)
                e_dl, r_dl = exp_bcast(lg_sb2, 1, f"b{g}")  # [128, j, 1, K]

            # ---- dl1 conv per sample ----
            build_ab_batched(p_dlb, e_dl, 0, ab_dl[:, sl])
            nc.vector.tensor_tensor(ab_dl[:, sl], ab_dl[:, sl], r_dl[:, :, 0], op=OP.mult)
            for j, s in enumerate(ss):
                matt = attp.tile([128, 2, K], F32, tag=f"matt{g}")
                for c in range(2):
                    nc.vector.tensor_scalar_mul(
                        matt[:, c, :], e_dl[:, j, 0, :], mask[:, c, s : s + 1]
                    )
                awd = awp.tile([128, 2, C1], CDT, tag=f"awd_{s}")
                for c in range(2):
                    eng = nc.gpsimd if (AWD_GP and c == 1) else nc.vector
                    eng.tensor_scalar_mul(
                        awd[:, c, :], p_dlw[:, c, 0, :], matt[:, c, 0:1]
                    )
                    for k in range(1, K):
                        eng.scalar_tensor_tensor(
                            awd[:, c, :], p_dlw[:, c, k, :], matt[:, c, k : k + 1],
                            awd[:, c, :], op0=OP.mult, op1=OP.add,
                        )

                # out in GROUPED spatial layout: col = h'*64 + parity*32 + w
                # (w' = 2w + parity); host un-interleaves.
                ot = outp.tile([128, HW2], F32, tag="out")
                for jj in range(4):
                    ps = bigps.tile([128, 1024], F32, tag="ps")
                    for half in range(2):
                        bank = ps[:, 512 * half : 512 * (half + 1)]
                        t = 2 * jj + half  # 512-block: h' rows 8t..8t+7
                        rhs0 = _ap(
                            X2U[s], 512 * t,
                            [list(X2U[s].ap[0]), [64, 8], [1, 2], [2, 32]],
                        )
                        nc.tensor.matmul(bank, awd[:, 0, :], rhs0, start=True, stop=False)
                        rhs1 = _ap(
                            L[s], 4 * t * 32,
                            [list(L[s].ap[0]), [32, 4], [0, 4], [1, 32]],
                        )
                        nc.tensor.matmul(bank, awd[:, 1, :], rhs1, start=False, stop=True)
                    nc.scalar.activation(
                        ot[:, 1024 * jj : 1024 * jj + 512], ps[:, 0:512],
                        AFT.Identity, bias=ab_dl[:, s : s + 1], scale=r_dl[:, j, 0:1],
                    )
                    nc.vector.tensor_scalar(
                        ot[:, 1024 * jj + 512 : 1024 * (jj + 1)],
                        ps[:, 512:1024], r_dl[:, j, 0:1], ab_dl[:, s : s + 1],
                        op0=OP.mult, op1=OP.add,
                    )
                    nc.sync.dma_start(
                        T["y"].ap()[s, :, 1024 * jj : 1024 * (jj + 1)],
                        ot[:, 1024 * jj : 1024 * (jj + 1)],
                    )


def _prep_params(i):
    """Host-side param preprocessing -> dict of DRAM arrays (shared by cores)."""
    f32 = np.float32
    bf = ml_dtypes.bfloat16

    def wT(w):  # [K, Co, Ci] -> [Ci//128, 128, K, Co]
        ci = w.shape[2]
        return np.ascontiguousarray(
            w.transpose(2, 0, 1).reshape(ci // 128, 128, K, w.shape[1])
        ).astype(bf)

    def fc1T(w, hid_pad):  # [Hid, C] -> [C//128, 128, hid_pad]
        c = w.shape[1]
        out = np.zeros((c // 128, 128, hid_pad), f32)
        out[:, :, : w.shape[0]] = w.T.reshape(c // 128, 128, w.shape[0])
        return out

    def fc2T(w, nh):  # [K, Hid] -> [nh, 128, K]
        out = np.zeros((nh, 128, K), f32)
        out.reshape(nh * 128, K)[: w.shape[1], :] = w.T
        return out

    fc2b = np.zeros((4, 3 * K), f32)
    fc2b[:, 0:K] = i["u2_fc2_b"][None, :]
    fc2b[:, K : 2 * K] = i["u1_fc2_b"][None, :]
    fc2b[:, 2 * K : 3 * K] = i["dl1_fc2_b"][None, :]

    return {
        "u2_wT": wT(i["u2_w"]),
        "u1_wT": wT(i["u1_w"]),
        "dl1_wT": wT(i["dl1_w"]),
        "u2_bT": np.ascontiguousarray(i["u2_b"].T).astype(f32),
        "u1_bT": np.ascontiguousarray(i["u1_b"].T).astype(f32),
        "dl1_bT": np.ascontiguousarray(i["dl1_b"].T).astype(f32),
        "u2_fc1T": fc1T(i["u2_fc1_w"], 256),
        "u1_fc1T": fc1T(i["u1_fc1_w"], 384),
        "dl1_fc1T": fc1T(i["dl1_fc1_w"], 384),
        "u2_fc2T": fc2T(i["u2_fc2_w"], 2),
        "u1_fc2T": fc2T(i["u1_fc2_w"], 3),
        "dl1_fc2T": fc2T(i["dl1_fc2_w"], 3),
        "fc2_b": fc2b,
        "ca_w1T": np.ascontiguousarray(i["ca_w1"].T.reshape(2, 128, C1)).astype(f32),
        "ca_w2T": np.ascontiguousarray(i["ca_w2"].T).astype(f32),
        "ca_b1": np.ascontiguousarray(i["ca_b1"][:, None]).astype(f32),
        # fus(avg)+fus(max) each add ca_b2 -> 2*ca_b2; negated because it is
        # applied as the bias of exp(-z - 2*ca_b2) in the sigmoid
        "ca_b2": np.ascontiguousarray(-2.0 * i["ca_b2"].reshape(2, 128)).astype(f32),
    }


def make_in_maps(**inputs):
    bf = ml_dtypes.bfloat16
    params = _prep_params(inputs)
    x2 = np.asarray(inputs["x2"]).reshape(B, C1, HW2).astype(bf)
    x3 = np.asarray(inputs["x3"]).reshape(B, 2, 128, HW3).astype(bf)
    in_maps = []
    for c in range(N_CORES):
        m = dict(params)
        m["x2"] = np.ascontiguousarray(x2[c * BL : (c + 1) * BL])
        m["x3"] = np.ascontiguousarray(x3[c * BL : (c + 1) * BL])
        in_maps.append(m)
    return in_maps


_NC_CACHE = None


def get_nc():
    global _NC_CACHE
    if _NC_CACHE is None:
        _NC_CACHE = build_nc()
    return _NC_CACHE


def unpack_out(y_cores):
    """y per core [BL, C1, HW2] in grouped layout (col = h'*64 + p*32 + w,
    w' = 2w + p) -> full [B, C1, 64, 64]."""
    out = np.concatenate(y_cores, axis=0).reshape(B, C1, 64, 2, 32)
    return np.ascontiguousarray(out.transpose(0, 1, 2, 4, 3).reshape(B, C1, 64, 64))


def kernel(**inputs):
    nc = get_nc()
    in_maps = make_in_maps(**inputs)
    res = run_bass_kernel_spmd(nc, in_maps, core_ids=list(range(N_CORES)))
    return unpack_out([res.results[c]["y"] for c in range(N_CORES)]).astype(np.float32)
